# revision 34
# baseline (speedup 1.0000x reference)
"""GQA attention kernel for Trainium2, 8 NeuronCores.

Sharding: data-parallel over batch (4) x tensor-parallel over head groups (2).
Each core handles one (batch, head-group): 8 query heads / 2 kv heads.
o_proj is row-parallel -> host sums the 2 partial outputs per batch.

v6 layout strategy (per core):
  - Inputs host-prepped: xT = x[b].T (bf16), weight shards (bf16),
    RoPE tables cosT/sinT [128, T] (bf16, sin sign-folded), causal masks.
  - Program order interleaves phases per 512-wide tile jt; the previous
    block's head-7 normalize tail + o_proj are deferred until after
    P1(jt+1) so the in-order PE stream never stalls on the tail chain.
  - P1: QT/KT via lhsT=w chunks (RoPE via SBUF shift-DMA + DVE, bf16);
    V in natural [T, dh] layout.
  - P2 per head: S^T in 2-chunk groups -> ONE exp per group on ScalarE
    ([128,1024] PSUM read amortizes the 352-cycle ACTIVATE overhead);
    0/1 mask on diagonal groups; O^T += V^T P^T; softmax denominator on
    VectorE (bf16 pair-add per group + f32 streamed tree per head;
    partition colsum via one ones-matmul); 1/den via exp(-ln(den)) on
    ScalarE (same activation table set as exp); broadcast via bf16
    outer-product matmul; normalize O^T on VectorE.
  - o_proj: lhsT=O^T slices, bf16 output; host sums partials in f32.
"""

import json as _json

import numpy as np
import ml_dtypes

import concourse.bass as bass
import concourse.mybir as mybir
import concourse.tile as tile

# --- walrus sync-wait legalizer -------------------------------------------
# The walrus build in this container encodes at most ONE sync-wait command
# per instruction ("Too many sync wait commands" in setupSyncWait<> for any
# instruction with 2+ waits, including Tile's own tail Drain). Legalize by
# splitting extra waits into standalone single-wait EventSemaphore
# instructions on the same engine, immediately before the instruction —
# identical semantics (the engine stalls on each wait in turn).

_MAX_WAITS = 1
_orig_to_json_bytes = bass.Bass.to_json_bytes


def _split_waits_json(raw: bytes) -> bytes:
    m = _json.loads(raw)
    changed = False
    for fn in m.get("functions", []):
        for bb in fn.get("blocks", []):
            out = []
            for inst in bb.get("instructions", []):
                si = inst.get("sync_info")
                waits = (si or {}).get("on_wait") or []
                if len(waits) > _MAX_WAITS:
                    changed = True
                    for k, w in enumerate(waits[:-_MAX_WAITS]):
                        out.append({
                            "debug": inst.get("debug", 0),
                            "engine": inst["engine"],
                            "ins": [], "outs": [],
                            "name": f"{inst['name']}-sw{k}",
                            "opcode": "EventSemaphore",
                            "sync_info": {"on_update": [], "on_wait": [w]},
                        })
                    si["on_wait"] = waits[-_MAX_WAITS:]
                out.append(inst)
            bb["instructions"] = out
    if not changed:
        return raw
    return _json.dumps(m).encode()


def _patched_to_json_bytes(self):
    return _split_waits_json(_orig_to_json_bytes(self))


bass.Bass.to_json_bytes = _patched_to_json_bytes
# --------------------------------------------------------------------------

B, D = 4, 2048
NH, NKV, HD = 16, 4, 128
NHL, NKVL = 8, 2          # per-core q heads / kv heads
DQ = NHL * HD             # 1024
DKV = NKVL * HD           # 256
KD = D // 128             # 16 contraction chunks
TQ = 512                  # query tile width (matmul free dim)
THETA = 10000.0
SCALE = HD ** -0.5
NCORES = 8

bf16 = mybir.dt.bfloat16
f32 = mybir.dt.float32
EXP = mybir.ActivationFunctionType.Exp
LN = mybir.ActivationFunctionType.Ln


def build_nc(T=2048):
    njq = T // TQ
    nck = T // 128
    ts = bass.ts

    nc = bass.Bass()
    xT = nc.dram_tensor("xT", [D, T], bf16, kind="ExternalInput")
    wq = nc.dram_tensor("wq", [D, DQ], bf16, kind="ExternalInput")
    wk = nc.dram_tensor("wk", [D, DKV], bf16, kind="ExternalInput")
    wv = nc.dram_tensor("wv", [D, DKV], bf16, kind="ExternalInput")
    wo = nc.dram_tensor("wo", [DQ, D], bf16, kind="ExternalInput")
    cosT = nc.dram_tensor("cosT", [HD, T], bf16, kind="ExternalInput")
    sinT = nc.dram_tensor("sinT", [HD, T], bf16, kind="ExternalInput")
    cmask = nc.dram_tensor("cmask", [128, 4, TQ], bf16, kind="ExternalInput")
    out = nc.dram_tensor("out", [T, D], bf16, kind="ExternalOutput")

    with tile.TileContext(nc) as tc:
        with tc.tile_pool(name="res", bufs=1) as res, \
             tc.tile_pool(name="wts", bufs=1) as wts, \
             tc.tile_pool(name="qt", bufs=2) as qtpool, \
             tc.tile_pool(name="xp", bufs=1) as xpool, \
             tc.tile_pool(name="tp", bufs=2) as tpool, \
             tc.tile_pool(name="gp", bufs=2, space="PSUM") as gp, \
             tc.tile_pool(name="sp", bufs=2, space="PSUM") as spool, \
             tc.tile_pool(name="op", bufs=2, space="PSUM") as opool, \
             tc.tile_pool(name="pp", bufs=3) as ppool, \
             tc.tile_pool(name="dg", bufs=3) as dgpool, \
             tc.tile_pool(name="tr", bufs=4) as trpool, \
             tc.tile_pool(name="sm", bufs=2) as smpool, \
             tc.tile_pool(name="oc", bufs=3) as ocpool, \
             tc.tile_pool(name="bc", bufs=2) as bcpool, \
             tc.tile_pool(name="ot", bufs=2) as otpool, \
             tc.tile_pool(name="ou", bufs=2) as outpool:

            KT_sb = res.tile([128, NKVL, T], bf16)
            V_sb = res.tile([128, nck, DKV], bf16)
            msk_sb = res.tile([128, 4, TQ], bf16)
            ones_bf = res.tile([128, 1], bf16)
            onesr_bf = res.tile([1, 128], bf16)
            dum_i = res.tile([1, 16], f32)
            dum_o = res.tile([1, 16], bf16)

            wq_lo = wts.tile([128, KD // 2, DQ], bf16)
            wq_hi = wts.tile([128, KD // 2, DQ], bf16)
            wk_sb = wts.tile([128, KD, DKV], bf16)
            wv_sb = wts.tile([128, KD, DKV], bf16)
            wo_sb = wts.tile([128, NHL, D], bf16)
            cos_sb = wts.tile([128, T], bf16)
            sin_sb = wts.tile([128, T], bf16)

            # preload the ln/exp table set right away (ACT is idle anyway)
            nc.vector.memset(dum_i, 1.0)
            nc.scalar.activation(dum_o, dum_i, EXP)
            # first-wave DMAs: what P1(jt=0)'s K/V heads need comes first
            nc.sync.dma_start(out=wk_sb, in_=wk[:, :].rearrange("(c p) m -> p c m", p=128))
            nc.sync.dma_start(out=wv_sb, in_=wv[:, :].rearrange("(c p) m -> p c m", p=128))
            nc.sync.dma_start(out=cos_sb, in_=cosT[:, :])
            nc.sync.dma_start(out=sin_sb, in_=sinT[:, :])
            nc.vector.memset(ones_bf, 1.0)
            nc.vector.memset(onesr_bf, 1.0)

            # Two-stage normalize tail, pipelined across heads so the PE
            # matmuls in it never wait on the ACT ln/exp chain.
            def emit_tail_a(o_ps, den_bf, ot_dst):
                """Free o_ps into SBUF, colsum the denominator, 1/den on ACT."""
                ocp = ocpool.tile([128, TQ], bf16, tag="ocp")
                nc.vector.tensor_copy(ocp, o_ps)
                den_ps = gp.tile([1, TQ], f32, tag="gp")
                nc.tensor.matmul(den_ps, lhsT=ones_bf, rhs=den_bf,
                                 start=True, stop=True)
                lden = smpool.tile([1, TQ], f32, tag="lden")
                nc.scalar.activation(lden, den_ps, LN)
                rden = smpool.tile([1, TQ], bf16, tag="rden")
                nc.scalar.activation(rden, lden, EXP, scale=-1.0)
                return (ocp, rden, ot_dst)

            def emit_tail_b(ocp, rden, ot_dst):
                """Broadcast 1/den across partitions (outer product), scale."""
                bc_ps = gp.tile([128, TQ], f32, tag="gp")
                nc.tensor.matmul(bc_ps, lhsT=onesr_bf, rhs=rden,
                                 start=True, stop=True)
                bc_sb = bcpool.tile([128, TQ], bf16, tag="bc")
                nc.vector.tensor_copy(bc_sb, bc_ps)
                nc.vector.tensor_mul(ot_dst, ocp, bc_sb)

            def emit_oproj(jq, OT):
                for s in range(4):
                    row = jq * TQ + s * 128
                    for half in range(2):
                        osb = outpool.tile([128, D // 2], bf16, tag="osb")
                        for nt in range(2):
                            ntg = half * 2 + nt
                            op_ps = gp.tile([128, TQ], f32, tag="gp")
                            for hc in range(NHL):
                                nc.tensor.matmul(
                                    op_ps,
                                    lhsT=OT[:, hc, s * 128:(s + 1) * 128],
                                    rhs=wo_sb[:, hc, ts(ntg, TQ)],
                                    start=(hc == 0), stop=(hc == NHL - 1))
                            nc.scalar.copy(osb[:, ts(nt, TQ)], op_ps)
                        nc.sync.dma_start(
                            out=out[row:row + 128, half * (D // 2):(half + 1) * (D // 2)],
                            in_=osb)

            def emit_p1_head(jt, h, QT):
                """Projection + RoPE for one head (h<NHL: Q, else K)."""
                if h < NHL:
                    col = h * 128
                    dst = QT[:, h, :]
                else:
                    g = h - NHL
                    col = g * 128
                    dst = KT_sb[:, g, ts(jt, TQ)]
                ps = gp.tile([128, TQ], f32, tag="gp")
                for c in range(KD):
                    if h < NHL:
                        w_sb = wq_lo if c < KD // 2 else wq_hi
                        ci = c % (KD // 2)
                    else:
                        w_sb, ci = wk_sb, c
                    nc.tensor.matmul(ps, lhsT=w_sb[:, ci, col:col + 128],
                                     rhs=xt_cur[0][:, c, :],
                                     start=(c == 0), stop=(c == KD - 1))
                qf = tpool.tile([128, TQ], bf16, tag="qf")
                nc.vector.tensor_copy(qf, ps)
                qs = tpool.tile([128, TQ], bf16, tag="qs")
                nc.sync.dma_start(out=qs[0:64, :], in_=qf[64:128, :])
                nc.sync.dma_start(out=qs[64:128, :], in_=qf[0:64, :])
                t1 = tpool.tile([128, TQ], bf16, tag="t1")
                nc.vector.tensor_mul(t1, qf, cos_sb[:, ts(jt, TQ)])
                nc.vector.tensor_mul(qs, qs, sin_sb[:, ts(jt, TQ)])
                nc.vector.tensor_add(dst, t1, qs)

            def emit_p1_v(jt, s):
                pv = gp.tile([128, TQ], f32, tag="gp")
                for c in range(KD):
                    nc.tensor.matmul(pv[:, 0:DKV],
                                     lhsT=xt_cur[0][:, c, s * 128:(s + 1) * 128],
                                     rhs=wv_sb[:, c, :],
                                     start=(c == 0), stop=(c == KD - 1))
                nc.vector.tensor_copy(V_sb[:, 4 * jt + s, :], pv[:, 0:DKV])

            xT_r = xT[:, :].rearrange("(c p) t -> p c t", p=128)
            xt_cur = [None]
            pend = None  # deferred (o_ps, den_bf, OT, jq) for head 7
            for jt in range(njq):
                # ---------------- Phase 1 for tile jt ----------------
                xt = xpool.tile([128, KD, TQ], bf16, tag="xt")
                xt_cur[0] = xt
                nc.sync.dma_start(out=xt_cur[0], in_=xT_r[:, :, ts(jt, TQ)])
                QT = qtpool.tile([128, NHL, TQ], bf16, tag="QT")
                wq_r = wq[:, :].rearrange("(c p) m -> p c m", p=128)
                if jt == 0:
                    # K/V first (need only wk/wv/xt); wq streams meanwhile
                    nc.sync.dma_start(out=wq_lo, in_=wq_r[:, 0:KD // 2, :])
                    nc.sync.dma_start(out=wq_hi, in_=wq_r[:, KD // 2:KD, :])
                    nc.sync.dma_start(out=msk_sb, in_=cmask[:, :, :])
                    for h in range(NHL, NHL + NKVL):
                        emit_p1_head(jt, h, QT)
                    for s in range(4):
                        emit_p1_v(jt, s)
                    for h in range(NHL):
                        emit_p1_head(jt, h, QT)
                    # wo only needed from o_proj(0), deferred to next block
                    nc.sync.dma_start(
                        out=wo_sb,
                        in_=wo[:, :].rearrange("(c p) n -> p c n", p=128))
                else:
                    for h in range(NHL + NKVL):
                        emit_p1_head(jt, h, QT)
                    for s in range(4):
                        emit_p1_v(jt, s)

                # deferred stage-B tails (heads 6,7) + o_proj of the
                # PREVIOUS block: their ACT deps completed during P1 above
                if pend is not None:
                    b6, b7, pot, pjq = pend
                    emit_tail_b(*b6)
                    emit_tail_b(*b7)
                    emit_oproj(pjq, pot)
                    pend = None

                # ---------------- Phase 2: attention for jq = jt -----
                jq = jt
                nchunks = 4 * jq + 4
                ngr = nchunks // 2
                OT = otpool.tile([128, NHL, TQ], bf16, tag="OT")
                penda = None  # head awaiting stage A (one-head deferral)
                pendb = None  # head awaiting stage B (two-head deferral)
                for h in range(NHL):
                    g = h // 4
                    o_ps = opool.tile([128, TQ], f32, tag="o")
                    den_run = None  # linear f32 accumulation of group sums
                    for gr in range(ngr):
                        s2 = spool.tile([128, 2, TQ], f32, tag="s")
                        for r in range(2):
                            c = 2 * gr + r
                            nc.tensor.matmul(
                                s2[:, r, :],
                                lhsT=KT_sb[:, g, c * 128:(c + 1) * 128],
                                rhs=QT[:, h, :],
                                start=True, stop=True)
                        p2 = ppool.tile([128, 2, TQ], bf16, tag="p")
                        nc.scalar.activation(p2, s2, EXP, scale=SCALE)
                        m = gr - 2 * jq
                        if m >= 0:  # diagonal-crossing group: 0/1 mask
                            nc.vector.tensor_mul(p2, p2,
                                                 msk_sb[:, 2 * m:2 * m + 2, :])
                        for r in range(2):
                            c = 2 * gr + r
                            nc.tensor.matmul(
                                o_ps,
                                lhsT=V_sb[:, c, g * 128:(g + 1) * 128],
                                rhs=p2[:, r, :],
                                start=(c == 0), stop=(c == nchunks - 1))
                        dgt = trpool.tile([128, TQ], f32, tag="tr")
                        nc.vector.tensor_add(dgt, p2[:, 0, :], p2[:, 1, :])
                        if den_run is None:
                            den_run = dgt
                        else:
                            nr = trpool.tile([128, TQ], f32, tag="tr")
                            nc.vector.tensor_add(nr, den_run, dgt)
                            den_run = nr
                    den_bf = dgpool.tile([128, TQ], bf16, tag="dg")
                    nc.vector.tensor_copy(den_bf, den_run)
                    if pendb is not None:
                        emit_tail_b(*pendb)
                        pendb = None
                    if penda is not None:
                        pendb = emit_tail_a(*penda)
                    penda = (o_ps, den_bf, OT[:, h, :])
                # flush: A for head 7 now (its DVE den tree is ~done); B for
                # heads 6,7 + o_proj deferred past the next block's P1
                b7 = emit_tail_a(*penda)
                pend = (pendb, b7, OT, jq)

            # last block's deferred stage-B tails + o_proj
            b6, b7, pot, pjq = pend
            emit_tail_b(*b6)
            emit_tail_b(*b7)
            emit_oproj(pjq, pot)
    return nc


def rope_tables(T=2048):
    inv = 1.0 / (THETA ** (np.arange(0, HD, 2, dtype=np.float32) / HD))
    t = np.arange(T, dtype=np.float32)
    freqs = np.outer(t, inv)
    emb = np.concatenate([freqs, freqs], -1)      # [T, 128]
    bf = ml_dtypes.bfloat16
    cos = np.ascontiguousarray(np.cos(emb).T.astype(bf))
    sin = np.sin(emb).T.astype(np.float32)
    sin_signed = sin.copy()
    sin_signed[:64] *= -1.0                        # rotate_half sign fold
    return cos, np.ascontiguousarray(sin_signed.astype(bf))


def causal_block_masks():
    k = np.arange(128)[:, None]
    q = np.arange(TQ)[None, :]
    cm = np.stack([(k + 128 * r) <= q for r in range(4)], axis=1)
    return np.ascontiguousarray(cm.astype(ml_dtypes.bfloat16))  # [128, 4, TQ]


def build_in_maps(x, wq, wk, wv, wo, T=2048):
    bf = ml_dtypes.bfloat16
    cos, sin_s = rope_tables(T)
    cm = causal_block_masks()
    wq16 = np.asarray(wq).astype(bf)
    wk16 = np.asarray(wk).astype(bf)
    wv16 = np.asarray(wv).astype(bf)
    wo16 = np.asarray(wo).astype(bf)
    in_maps = []
    for core in range(NCORES):
        b, hg = core // 2, core % 2
        in_maps.append({
            "xT": np.ascontiguousarray(np.asarray(x)[b].T).astype(bf),
            "wq": np.ascontiguousarray(wq16[:, hg * DQ:(hg + 1) * DQ]),
            "wk": np.ascontiguousarray(wk16[:, hg * DKV:(hg + 1) * DKV]),
            "wv": np.ascontiguousarray(wv16[:, hg * DKV:(hg + 1) * DKV]),
            "wo": np.ascontiguousarray(wo16[hg * DQ:(hg + 1) * DQ, :]),
            "cosT": cos, "sinT": sin_s, "cmask": cm,
        })
    return in_maps


_NC_CACHE = {}


def get_nc(T=2048):
    if T not in _NC_CACHE:
        _NC_CACHE[T] = build_nc(T)
    return _NC_CACHE[T]


def run(inputs, trace=False, **kw):
    """Returns (full_output [B,T,D] f32, BassKernelResults)."""
    from concourse import bass_utils
    x = np.asarray(inputs["x"], dtype=np.float32)
    T = x.shape[1]
    nc = get_nc(T)
    in_maps = build_in_maps(x, inputs["wq"], inputs["wk"], inputs["wv"],
                            inputs["wo"], T)
    res = bass_utils.run_bass_kernel_spmd(nc, in_maps,
                                          core_ids=list(range(NCORES)),
                                          trace=trace, **kw)
    outs = [np.asarray(r["out"]) for r in res.results]
    full = np.empty((B, T, D), dtype=np.float32)
    for b in range(B):
        full[b] = outs[2 * b].astype(np.float32) + outs[2 * b + 1].astype(np.float32)
    return full, res


def kernel(x, mask, wq, wk, wv, wo):
    full, _ = run({"x": x, "mask": mask, "wq": wq, "wk": wk, "wv": wv, "wo": wo})
    return full


# revision 38
# speedup vs baseline: 1.1483x; 1.1483x over previous
"""GQA attention kernel for Trainium2, 8 NeuronCores.

Sharding: data-parallel over batch (4) x tensor-parallel over head groups (2).
Each core handles one (batch, head-group): 8 query heads / 2 kv heads.
o_proj is row-parallel -> host sums the 2 partial outputs per batch.

v7 layout strategy (per core):
  - Inputs host-prepped: xT = x[b].T (bf16), weight shards (bf16),
    RoPE tables cosT/sinT [128, T] (bf16, sin sign-folded), causal masks.
  - Program order interleaves phases per 512-wide tile jt; the in-order
    PE stream never waits on ACT/DVE chains: each head's normalize tail
    is split in two stages pipelined across heads, and the last heads'
    tails + o_proj are deferred past the next block's P1.
  - P1: QT/KT via lhsT=w chunks (RoPE via SBUF shift-DMA + DVE, bf16);
    V in natural [T, dh] layout.
  - P2 per head, per 128-k chunk: S^T = K Q^T (PE); exp on ScalarE
    (PSUM->SBUF bf16, scaled); 0/1 mask on diagonal chunks (DVE);
    O^T += V^T P^T and denominator += ones^T P^T (PE, 3 PSUM banks).
    Tail stage A (one head later): unnormalized O^T to SBUF (DVE),
    1/den = exp(-ln(den)) on ScalarE (same table set as exp).
    Tail stage B (two heads later): broadcast 1/den via bf16 outer
    product (PE), scale O^T (DVE).
  - o_proj: lhsT=O^T slices, bf16 output; host sums partials in f32.
"""

import json as _json

import numpy as np
import ml_dtypes

import concourse.bass as bass
import concourse.mybir as mybir
import concourse.tile as tile

# --- walrus sync-wait legalizer -------------------------------------------
# The walrus build in this container encodes at most ONE sync-wait command
# per instruction ("Too many sync wait commands" in setupSyncWait<> for any
# instruction with 2+ waits, including Tile's own tail Drain). Legalize by
# splitting extra waits into standalone single-wait EventSemaphore
# instructions on the same engine, immediately before the instruction —
# identical semantics (the engine stalls on each wait in turn).

_MAX_WAITS = 1
_orig_to_json_bytes = bass.Bass.to_json_bytes


def _split_waits_json(raw: bytes) -> bytes:
    m = _json.loads(raw)
    changed = False
    for fn in m.get("functions", []):
        for bb in fn.get("blocks", []):
            out = []
            for inst in bb.get("instructions", []):
                si = inst.get("sync_info")
                waits = (si or {}).get("on_wait") or []
                if len(waits) > _MAX_WAITS:
                    changed = True
                    for k, w in enumerate(waits[:-_MAX_WAITS]):
                        out.append({
                            "debug": inst.get("debug", 0),
                            "engine": inst["engine"],
                            "ins": [], "outs": [],
                            "name": f"{inst['name']}-sw{k}",
                            "opcode": "EventSemaphore",
                            "sync_info": {"on_update": [], "on_wait": [w]},
                        })
                    si["on_wait"] = waits[-_MAX_WAITS:]
                out.append(inst)
            bb["instructions"] = out
    if not changed:
        return raw
    return _json.dumps(m).encode()


def _patched_to_json_bytes(self):
    return _split_waits_json(_orig_to_json_bytes(self))


bass.Bass.to_json_bytes = _patched_to_json_bytes
# --------------------------------------------------------------------------

B, D = 4, 2048
NH, NKV, HD = 16, 4, 128
NHL, NKVL = 8, 2          # per-core q heads / kv heads
DQ = NHL * HD             # 1024
DKV = NKVL * HD           # 256
KD = D // 128             # 16 contraction chunks
TQ = 512                  # query tile width (matmul free dim)
THETA = 10000.0
SCALE = HD ** -0.5
NCORES = 8

bf16 = mybir.dt.bfloat16
f32 = mybir.dt.float32
EXP = mybir.ActivationFunctionType.Exp
LN = mybir.ActivationFunctionType.Ln


def build_nc(T=2048):
    njq = T // TQ
    nck = T // 128
    ts = bass.ts

    nc = bass.Bass()
    xT = nc.dram_tensor("xT", [D, T], bf16, kind="ExternalInput")
    wq = nc.dram_tensor("wq", [D, DQ], bf16, kind="ExternalInput")
    wk = nc.dram_tensor("wk", [D, DKV], bf16, kind="ExternalInput")
    wv = nc.dram_tensor("wv", [D, DKV], bf16, kind="ExternalInput")
    wo = nc.dram_tensor("wo", [DQ, D], bf16, kind="ExternalInput")
    cosT = nc.dram_tensor("cosT", [HD, T], bf16, kind="ExternalInput")
    sinT = nc.dram_tensor("sinT", [HD, T], bf16, kind="ExternalInput")
    cmask = nc.dram_tensor("cmask", [128, 4, TQ], bf16, kind="ExternalInput")
    out = nc.dram_tensor("out", [T, D], bf16, kind="ExternalOutput")

    with tile.TileContext(nc) as tc:
        with tc.tile_pool(name="res", bufs=1) as res, \
             tc.tile_pool(name="wts", bufs=1) as wts, \
             tc.tile_pool(name="qt", bufs=2) as qtpool, \
             tc.tile_pool(name="xp", bufs=1) as xpool, \
             tc.tile_pool(name="tp", bufs=2) as tpool, \
             tc.tile_pool(name="gp", bufs=2, space="PSUM") as gp, \
             tc.tile_pool(name="sp", bufs=2, space="PSUM") as spool, \
             tc.tile_pool(name="op", bufs=2, space="PSUM") as opool, \
             tc.tile_pool(name="nrm", bufs=2, space="PSUM") as nrm, \
             tc.tile_pool(name="pp", bufs=4) as ppool, \
             tc.tile_pool(name="sm", bufs=2) as smpool, \
             tc.tile_pool(name="oc", bufs=3) as ocpool, \
             tc.tile_pool(name="bc", bufs=2) as bcpool, \
             tc.tile_pool(name="ot", bufs=2) as otpool, \
             tc.tile_pool(name="ou", bufs=2) as outpool:

            KT_sb = res.tile([128, NKVL, T], bf16)
            V_sb = res.tile([128, nck, DKV], bf16)
            msk_sb = res.tile([128, 4, TQ], bf16)
            ones_bf = res.tile([128, 1], bf16)
            onesr_bf = res.tile([1, 128], bf16)
            dum_i = res.tile([1, 16], f32)
            dum_o = res.tile([1, 16], bf16)

            wq_lo = wts.tile([128, KD // 2, DQ], bf16)
            wq_hi = wts.tile([128, KD // 2, DQ], bf16)
            wk_sb = wts.tile([128, KD, DKV], bf16)
            wv_sb = wts.tile([128, KD, DKV], bf16)
            wo_sb = wts.tile([128, NHL, D], bf16)
            cos_sb = wts.tile([128, T], bf16)
            sin_sb = wts.tile([128, T], bf16)

            # preload the ln/exp table set right away (ACT is idle anyway)
            nc.vector.memset(dum_i, 1.0)
            nc.scalar.activation(dum_o, dum_i, EXP)
            # first-wave DMAs: what P1(jt=0)'s K/V heads need comes first
            nc.sync.dma_start(out=wk_sb, in_=wk[:, :].rearrange("(c p) m -> p c m", p=128))
            nc.sync.dma_start(out=wv_sb, in_=wv[:, :].rearrange("(c p) m -> p c m", p=128))
            nc.sync.dma_start(out=cos_sb, in_=cosT[:, :])
            nc.sync.dma_start(out=sin_sb, in_=sinT[:, :])
            nc.vector.memset(ones_bf, 1.0)
            nc.vector.memset(onesr_bf, 1.0)

            # Two-stage normalize tail, pipelined across heads so the PE
            # matmuls in it never wait on the ACT ln/exp chain.
            def emit_tail_a(o_ps, den_ps, ot_dst):
                """Free o_ps into SBUF; 1/den via exp(-ln) on ACT."""
                ocp = ocpool.tile([128, TQ], bf16, tag="ocp")
                nc.vector.tensor_copy(ocp, o_ps)
                lden = smpool.tile([1, TQ], f32, tag="lden")
                nc.scalar.activation(lden, den_ps, LN)
                rden = smpool.tile([1, TQ], bf16, tag="rden")
                nc.scalar.activation(rden, lden, EXP, scale=-1.0)
                return (ocp, rden, ot_dst)

            def emit_tail_b(ocp, rden, ot_dst):
                """Broadcast 1/den across partitions (outer product), scale."""
                bc_ps = gp.tile([128, TQ], f32, tag="gp")
                nc.tensor.matmul(bc_ps, lhsT=onesr_bf, rhs=rden,
                                 start=True, stop=True)
                bc_sb = bcpool.tile([128, TQ], bf16, tag="bc")
                nc.vector.tensor_copy(bc_sb, bc_ps)
                nc.vector.tensor_mul(ot_dst, ocp, bc_sb)

            def emit_oproj(jq, OT):
                for s in range(4):
                    row = jq * TQ + s * 128
                    for half in range(2):
                        osb = outpool.tile([128, D // 2], bf16, tag="osb")
                        for nt in range(2):
                            ntg = half * 2 + nt
                            op_ps = gp.tile([128, TQ], f32, tag="gp")
                            for hc in range(NHL):
                                nc.tensor.matmul(
                                    op_ps,
                                    lhsT=OT[:, hc, s * 128:(s + 1) * 128],
                                    rhs=wo_sb[:, hc, ts(ntg, TQ)],
                                    start=(hc == 0), stop=(hc == NHL - 1))
                            nc.scalar.copy(osb[:, ts(nt, TQ)], op_ps)
                        nc.sync.dma_start(
                            out=out[row:row + 128, half * (D // 2):(half + 1) * (D // 2)],
                            in_=osb)

            def emit_p1_head(jt, h, QT):
                """Projection + RoPE for one head (h<NHL: Q, else K)."""
                if h < NHL:
                    col = h * 128
                    dst = QT[:, h, :]
                else:
                    g = h - NHL
                    col = g * 128
                    dst = KT_sb[:, g, ts(jt, TQ)]
                ps = gp.tile([128, TQ], f32, tag="gp")
                for c in range(KD):
                    if h < NHL:
                        w_sb = wq_lo if c < KD // 2 else wq_hi
                        ci = c % (KD // 2)
                    else:
                        w_sb, ci = wk_sb, c
                    nc.tensor.matmul(ps, lhsT=w_sb[:, ci, col:col + 128],
                                     rhs=xt_cur[0][:, c, :],
                                     start=(c == 0), stop=(c == KD - 1))
                qf = tpool.tile([128, TQ], bf16, tag="qf")
                nc.vector.tensor_copy(qf, ps)
                qs = tpool.tile([128, TQ], bf16, tag="qs")
                nc.sync.dma_start(out=qs[0:64, :], in_=qf[64:128, :])
                nc.sync.dma_start(out=qs[64:128, :], in_=qf[0:64, :])
                t1 = tpool.tile([128, TQ], bf16, tag="t1")
                nc.vector.tensor_mul(t1, qf, cos_sb[:, ts(jt, TQ)])
                nc.vector.tensor_mul(qs, qs, sin_sb[:, ts(jt, TQ)])
                nc.vector.tensor_add(dst, t1, qs)

            def emit_p1_v(jt, s):
                pv = gp.tile([128, TQ], f32, tag="gp")
                for c in range(KD):
                    nc.tensor.matmul(pv[:, 0:DKV],
                                     lhsT=xt_cur[0][:, c, s * 128:(s + 1) * 128],
                                     rhs=wv_sb[:, c, :],
                                     start=(c == 0), stop=(c == KD - 1))
                nc.vector.tensor_copy(V_sb[:, 4 * jt + s, :], pv[:, 0:DKV])

            xT_r = xT[:, :].rearrange("(c p) t -> p c t", p=128)
            xt_cur = [None]
            pend = None  # deferred (o_ps, den_bf, OT, jq) for head 7
            for jt in range(njq):
                # ---------------- Phase 1 for tile jt ----------------
                xt = xpool.tile([128, KD, TQ], bf16, tag="xt")
                xt_cur[0] = xt
                nc.sync.dma_start(out=xt_cur[0], in_=xT_r[:, :, ts(jt, TQ)])
                QT = qtpool.tile([128, NHL, TQ], bf16, tag="QT")
                wq_r = wq[:, :].rearrange("(c p) m -> p c m", p=128)
                if jt == 0:
                    # K/V first (need only wk/wv/xt); wq streams meanwhile
                    nc.sync.dma_start(out=wq_lo, in_=wq_r[:, 0:KD // 2, :])
                    nc.sync.dma_start(out=wq_hi, in_=wq_r[:, KD // 2:KD, :])
                    nc.sync.dma_start(out=msk_sb, in_=cmask[:, :, :])
                    for h in range(NHL, NHL + NKVL):
                        emit_p1_head(jt, h, QT)
                    for s in range(4):
                        emit_p1_v(jt, s)
                    for h in range(NHL):
                        emit_p1_head(jt, h, QT)
                    # wo only needed from o_proj(0), deferred to next block
                    nc.sync.dma_start(
                        out=wo_sb,
                        in_=wo[:, :].rearrange("(c p) n -> p c n", p=128))
                else:
                    for h in range(NHL + NKVL):
                        emit_p1_head(jt, h, QT)
                    for s in range(4):
                        emit_p1_v(jt, s)

                # deferred stage-B tails (heads 6,7) + o_proj of the
                # PREVIOUS block: their ACT deps completed during P1 above
                if pend is not None:
                    b6, b7, pot, pjq = pend
                    emit_tail_b(*b6)
                    emit_tail_b(*b7)
                    emit_oproj(pjq, pot)
                    pend = None

                # ---------------- Phase 2: attention for jq = jt -----
                jq = jt
                nchunks = 4 * jq + 4
                OT = otpool.tile([128, NHL, TQ], bf16, tag="OT")
                penda = None  # head awaiting stage A (one-head deferral)
                pendb = None  # head awaiting stage B (two-head deferral)
                for h in range(NHL):
                    g = h // 4
                    o_ps = opool.tile([128, TQ], f32, tag="o")
                    den_ps = nrm.tile([1, TQ], f32, tag="nrm")
                    for c in range(nchunks):
                        s_ps = spool.tile([128, TQ], f32, tag="s")
                        nc.tensor.matmul(s_ps,
                                         lhsT=KT_sb[:, g, c * 128:(c + 1) * 128],
                                         rhs=QT[:, h, :],
                                         start=True, stop=True)
                        p = ppool.tile([128, TQ], bf16, tag="p")
                        nc.scalar.activation(p, s_ps, EXP, scale=SCALE)
                        r = c - 4 * jq
                        if r >= 0:  # diagonal-crossing chunk: 0/1 mask
                            nc.vector.tensor_mul(p, p, msk_sb[:, r, :])
                        nc.tensor.matmul(o_ps,
                                         lhsT=V_sb[:, c, g * 128:(g + 1) * 128],
                                         rhs=p,
                                         start=(c == 0), stop=(c == nchunks - 1))
                        nc.tensor.matmul(den_ps, lhsT=ones_bf, rhs=p,
                                         start=(c == 0), stop=(c == nchunks - 1))
                    if pendb is not None:
                        emit_tail_b(*pendb)
                        pendb = None
                    if penda is not None:
                        pendb = emit_tail_a(*penda)
                    penda = (o_ps, den_ps, OT[:, h, :])
                # flush: A for head 7 now; B for heads 6,7 + o_proj are
                # deferred past the next block's P1
                b7 = emit_tail_a(*penda)
                pend = (pendb, b7, OT, jq)

            # last block's deferred stage-B tails + o_proj
            b6, b7, pot, pjq = pend
            emit_tail_b(*b6)
            emit_tail_b(*b7)
            emit_oproj(pjq, pot)
    return nc


def rope_tables(T=2048):
    inv = 1.0 / (THETA ** (np.arange(0, HD, 2, dtype=np.float32) / HD))
    t = np.arange(T, dtype=np.float32)
    freqs = np.outer(t, inv)
    emb = np.concatenate([freqs, freqs], -1)      # [T, 128]
    bf = ml_dtypes.bfloat16
    cos = np.ascontiguousarray(np.cos(emb).T.astype(bf))
    sin = np.sin(emb).T.astype(np.float32)
    sin_signed = sin.copy()
    sin_signed[:64] *= -1.0                        # rotate_half sign fold
    return cos, np.ascontiguousarray(sin_signed.astype(bf))


def causal_block_masks():
    k = np.arange(128)[:, None]
    q = np.arange(TQ)[None, :]
    cm = np.stack([(k + 128 * r) <= q for r in range(4)], axis=1)
    return np.ascontiguousarray(cm.astype(ml_dtypes.bfloat16))  # [128, 4, TQ]


def build_in_maps(x, wq, wk, wv, wo, T=2048):
    bf = ml_dtypes.bfloat16
    cos, sin_s = rope_tables(T)
    cm = causal_block_masks()
    wq16 = np.asarray(wq).astype(bf)
    wk16 = np.asarray(wk).astype(bf)
    wv16 = np.asarray(wv).astype(bf)
    wo16 = np.asarray(wo).astype(bf)
    in_maps = []
    for core in range(NCORES):
        b, hg = core // 2, core % 2
        in_maps.append({
            "xT": np.ascontiguousarray(np.asarray(x)[b].T).astype(bf),
            "wq": np.ascontiguousarray(wq16[:, hg * DQ:(hg + 1) * DQ]),
            "wk": np.ascontiguousarray(wk16[:, hg * DKV:(hg + 1) * DKV]),
            "wv": np.ascontiguousarray(wv16[:, hg * DKV:(hg + 1) * DKV]),
            "wo": np.ascontiguousarray(wo16[hg * DQ:(hg + 1) * DQ, :]),
            "cosT": cos, "sinT": sin_s, "cmask": cm,
        })
    return in_maps


_NC_CACHE = {}


def get_nc(T=2048):
    if T not in _NC_CACHE:
        _NC_CACHE[T] = build_nc(T)
    return _NC_CACHE[T]


def run(inputs, trace=False, **kw):
    """Returns (full_output [B,T,D] f32, BassKernelResults)."""
    from concourse import bass_utils
    x = np.asarray(inputs["x"], dtype=np.float32)
    T = x.shape[1]
    nc = get_nc(T)
    in_maps = build_in_maps(x, inputs["wq"], inputs["wk"], inputs["wv"],
                            inputs["wo"], T)
    res = bass_utils.run_bass_kernel_spmd(nc, in_maps,
                                          core_ids=list(range(NCORES)),
                                          trace=trace, **kw)
    outs = [np.asarray(r["out"]) for r in res.results]
    full = np.empty((B, T, D), dtype=np.float32)
    for b in range(B):
        full[b] = outs[2 * b].astype(np.float32) + outs[2 * b + 1].astype(np.float32)
    return full, res


def kernel(x, mask, wq, wk, wv, wo):
    full, _ = run({"x": x, "mask": mask, "wq": wq, "wk": wk, "wv": wv, "wo": wo})
    return full


# revision 42
# speedup vs baseline: 1.2049x; 1.0493x over previous
"""GQA attention kernel for Trainium2, 8 NeuronCores.

Sharding: data-parallel over batch (4) x tensor-parallel over head groups (2).
Each core handles one (batch, head-group): 8 query heads / 2 kv heads.
o_proj is row-parallel -> host sums the 2 partial outputs per batch.

v7 layout strategy (per core):
  - Inputs host-prepped: xT = x[b].T (bf16), weight shards (bf16),
    RoPE tables cosT/sinT [128, T] (bf16, sin sign-folded), causal masks.
  - Program order interleaves phases per 512-wide tile jt; the in-order
    PE stream never waits on ACT/DVE chains: each head's normalize tail
    is split in two stages pipelined across heads, and the last heads'
    tails + o_proj are deferred past the next block's P1.
  - P1: QT/KT via lhsT=w chunks (RoPE via SBUF shift-DMA + DVE, bf16);
    V in natural [T, dh] layout.
  - P2 per head, per 128-k chunk: S^T = K Q^T (PE); exp on ScalarE
    (PSUM->SBUF bf16, scaled); 0/1 mask on diagonal chunks (DVE);
    O^T += V^T P^T and denominator += ones^T P^T (PE, 3 PSUM banks).
    Tail stage A (one head later): unnormalized O^T to SBUF (DVE),
    1/den = exp(-ln(den)) on ScalarE (same table set as exp).
    Tail stage B (two heads later): broadcast 1/den via bf16 outer
    product (PE), scale O^T (DVE).
  - o_proj: lhsT=O^T slices, bf16 output; host sums partials in f32.
"""

import json as _json

import numpy as np
import ml_dtypes

import concourse.bass as bass
import concourse.mybir as mybir
import concourse.tile as tile

# --- walrus sync-wait legalizer -------------------------------------------
# The walrus build in this container encodes at most ONE sync-wait command
# per instruction ("Too many sync wait commands" in setupSyncWait<> for any
# instruction with 2+ waits, including Tile's own tail Drain). Legalize by
# splitting extra waits into standalone single-wait EventSemaphore
# instructions on the same engine, immediately before the instruction —
# identical semantics (the engine stalls on each wait in turn).

_MAX_WAITS = 1
_orig_to_json_bytes = bass.Bass.to_json_bytes


def _split_waits_json(raw: bytes) -> bytes:
    m = _json.loads(raw)
    changed = False
    for fn in m.get("functions", []):
        for bb in fn.get("blocks", []):
            out = []
            for inst in bb.get("instructions", []):
                si = inst.get("sync_info")
                waits = (si or {}).get("on_wait") or []
                if len(waits) > _MAX_WAITS:
                    changed = True
                    for k, w in enumerate(waits[:-_MAX_WAITS]):
                        out.append({
                            "debug": inst.get("debug", 0),
                            "engine": inst["engine"],
                            "ins": [], "outs": [],
                            "name": f"{inst['name']}-sw{k}",
                            "opcode": "EventSemaphore",
                            "sync_info": {"on_update": [], "on_wait": [w]},
                        })
                    si["on_wait"] = waits[-_MAX_WAITS:]
                out.append(inst)
            bb["instructions"] = out
    if not changed:
        return raw
    return _json.dumps(m).encode()


def _patched_to_json_bytes(self):
    return _split_waits_json(_orig_to_json_bytes(self))


bass.Bass.to_json_bytes = _patched_to_json_bytes
# --------------------------------------------------------------------------

B, D = 4, 2048
NH, NKV, HD = 16, 4, 128
NHL, NKVL = 8, 2          # per-core q heads / kv heads
DQ = NHL * HD             # 1024
DKV = NKVL * HD           # 256
KD = D // 128             # 16 contraction chunks
TQ = 512                  # query tile width (matmul free dim)
THETA = 10000.0
SCALE = HD ** -0.5
NCORES = 8

bf16 = mybir.dt.bfloat16
f32 = mybir.dt.float32
EXP = mybir.ActivationFunctionType.Exp
LN = mybir.ActivationFunctionType.Ln


def build_nc(T=2048):
    njq = T // TQ
    nck = T // 128
    ts = bass.ts

    nc = bass.Bass()
    xT = nc.dram_tensor("xT", [D, T], bf16, kind="ExternalInput")
    wq = nc.dram_tensor("wq", [D, DQ], bf16, kind="ExternalInput")
    wk = nc.dram_tensor("wk", [D, DKV], bf16, kind="ExternalInput")
    wv = nc.dram_tensor("wv", [D, DKV], bf16, kind="ExternalInput")
    wo = nc.dram_tensor("wo", [DQ, D], bf16, kind="ExternalInput")
    cosT = nc.dram_tensor("cosT", [HD, T], bf16, kind="ExternalInput")
    sinT = nc.dram_tensor("sinT", [HD, T], bf16, kind="ExternalInput")
    cmask = nc.dram_tensor("cmask", [128, 4, TQ], bf16, kind="ExternalInput")
    out = nc.dram_tensor("out", [T, D], bf16, kind="ExternalOutput")

    with tile.TileContext(nc) as tc:
        with tc.tile_pool(name="res", bufs=1) as res, \
             tc.tile_pool(name="wts", bufs=1) as wts, \
             tc.tile_pool(name="qt", bufs=2) as qtpool, \
             tc.tile_pool(name="xp", bufs=1) as xpool, \
             tc.tile_pool(name="tp", bufs=2) as tpool, \
             tc.tile_pool(name="gp", bufs=2, space="PSUM") as gp, \
             tc.tile_pool(name="sp", bufs=2, space="PSUM") as spool, \
             tc.tile_pool(name="op", bufs=2, space="PSUM") as opool, \
             tc.tile_pool(name="nrm", bufs=2, space="PSUM") as nrm, \
             tc.tile_pool(name="pp", bufs=4) as ppool, \
             tc.tile_pool(name="sm", bufs=2) as smpool, \
             tc.tile_pool(name="oc", bufs=3) as ocpool, \
             tc.tile_pool(name="bc", bufs=2) as bcpool, \
             tc.tile_pool(name="ot", bufs=2) as otpool, \
             tc.tile_pool(name="ou", bufs=2) as outpool:

            KT_sb = res.tile([128, NKVL, T], bf16)
            V_sb = res.tile([128, nck, DKV], bf16)
            msk_sb = res.tile([128, 4, TQ], bf16)
            ones_bf = res.tile([128, 1], bf16)
            onesr_bf = res.tile([1, 128], bf16)
            dum_i = res.tile([1, 16], f32)
            dum_o = res.tile([1, 16], bf16)

            wq_lo = wts.tile([128, KD // 2, DQ], bf16)
            wq_hi = wts.tile([128, KD // 2, DQ], bf16)
            wk_sb = wts.tile([128, KD, DKV], bf16)
            wv_sb = wts.tile([128, KD, DKV], bf16)
            wo_sb = wts.tile([128, NHL, D], bf16)
            cos_sb = wts.tile([128, T], bf16)
            sin_sb = wts.tile([128, T], bf16)

            # preload the ln/exp table set right away (ACT is idle anyway)
            nc.vector.memset(dum_i, 1.0)
            nc.scalar.activation(dum_o, dum_i, EXP)
            # first-wave DMAs: what P1(jt=0)'s K/V heads need comes first
            nc.sync.dma_start(out=wk_sb, in_=wk[:, :].rearrange("(c p) m -> p c m", p=128))
            nc.sync.dma_start(out=wv_sb, in_=wv[:, :].rearrange("(c p) m -> p c m", p=128))
            nc.sync.dma_start(out=cos_sb, in_=cosT[:, :])
            nc.sync.dma_start(out=sin_sb, in_=sinT[:, :])
            nc.vector.memset(ones_bf, 1.0)
            nc.vector.memset(onesr_bf, 1.0)

            # Two-stage normalize tail, pipelined across heads so the PE
            # matmuls in it never wait on the ACT ln/exp chain.
            def emit_tail_a(o_ps, den_ps, ot_dst):
                """Free o_ps into SBUF; 1/den via exp(-ln) on ACT."""
                ocp = ocpool.tile([128, TQ], bf16, tag="ocp")
                nc.vector.tensor_copy(ocp, o_ps)
                lden = smpool.tile([1, TQ], f32, tag="lden")
                nc.scalar.activation(lden, den_ps, LN)
                rden = smpool.tile([1, TQ], bf16, tag="rden")
                nc.scalar.activation(rden, lden, EXP, scale=-1.0)
                return (ocp, rden, ot_dst)

            def emit_tail_b(ocp, rden, ot_dst):
                """Broadcast 1/den across partitions (outer product), scale."""
                bc_ps = gp.tile([128, TQ], f32, tag="gp")
                nc.tensor.matmul(bc_ps, lhsT=onesr_bf, rhs=rden,
                                 start=True, stop=True)
                bc_sb = bcpool.tile([128, TQ], bf16, tag="bc")
                nc.vector.tensor_copy(bc_sb, bc_ps)
                nc.vector.tensor_mul(ot_dst, ocp, bc_sb)

            def emit_oproj(jq, OT):
                for s in range(4):
                    row = jq * TQ + s * 128
                    for half in range(2):
                        osb = outpool.tile([128, D // 2], bf16, tag="osb")
                        for nt in range(2):
                            ntg = half * 2 + nt
                            op_ps = gp.tile([128, TQ], f32, tag="gp")
                            for hc in range(NHL):
                                nc.tensor.matmul(
                                    op_ps,
                                    lhsT=OT[:, hc, s * 128:(s + 1) * 128],
                                    rhs=wo_sb[:, hc, ts(ntg, TQ)],
                                    start=(hc == 0), stop=(hc == NHL - 1))
                            nc.scalar.copy(osb[:, ts(nt, TQ)], op_ps)
                        nc.sync.dma_start(
                            out=out[row:row + 128, half * (D // 2):(half + 1) * (D // 2)],
                            in_=osb)

            def emit_p1_head(jt, h, QT):
                """Projection + RoPE for one head (h<NHL: Q, else K)."""
                if h < NHL:
                    col = h * 128
                    dst = QT[:, h, :]
                else:
                    g = h - NHL
                    col = g * 128
                    dst = KT_sb[:, g, ts(jt, TQ)]
                ps = gp.tile([128, TQ], f32, tag="gp")
                for c in range(KD):
                    if h < NHL:
                        w_sb = wq_lo if c < KD // 2 else wq_hi
                        ci = c % (KD // 2)
                    else:
                        w_sb, ci = wk_sb, c
                    nc.tensor.matmul(ps, lhsT=w_sb[:, ci, col:col + 128],
                                     rhs=xt_cur[c // 4][:, c % 4, :],
                                     start=(c == 0), stop=(c == KD - 1))
                qf = tpool.tile([128, TQ], bf16, tag="qf")
                nc.vector.tensor_copy(qf, ps)
                qs = tpool.tile([128, TQ], bf16, tag="qs")
                nc.sync.dma_start(out=qs[0:64, :], in_=qf[64:128, :])
                nc.sync.dma_start(out=qs[64:128, :], in_=qf[0:64, :])
                t1 = tpool.tile([128, TQ], bf16, tag="t1")
                nc.vector.tensor_mul(t1, qf, cos_sb[:, ts(jt, TQ)])
                nc.vector.tensor_mul(qs, qs, sin_sb[:, ts(jt, TQ)])
                nc.vector.tensor_add(dst, t1, qs)

            def emit_p1_v(jt, s):
                pv = gp.tile([128, TQ], f32, tag="gp")
                for c in range(KD):
                    nc.tensor.matmul(pv[:, 0:DKV],
                                     lhsT=xt_cur[c // 4][:, c % 4, s * 128:(s + 1) * 128],
                                     rhs=wv_sb[:, c, :],
                                     start=(c == 0), stop=(c == KD - 1))
                nc.vector.tensor_copy(V_sb[:, 4 * jt + s, :], pv[:, 0:DKV])

            xT_r = xT[:, :].rearrange("(c p) t -> p c t", p=128)
            xt_cur = [None] * 4
            pend = None  # deferred stage-B tails + o_proj carry
            for jt in range(njq):
                # ---------------- Phase 1 for tile jt ----------------
                # xt in 4 sub-tiles: the first matmul only waits on 512 KB
                for q4 in range(4):
                    xq = xpool.tile([128, 4, TQ], bf16, tag=f"xt{q4}")
                    xt_cur[q4] = xq
                    nc.sync.dma_start(
                        out=xq, in_=xT_r[:, 4 * q4:4 * q4 + 4, ts(jt, TQ)])
                QT = qtpool.tile([128, NHL, TQ], bf16, tag="QT")
                wq_r = wq[:, :].rearrange("(c p) m -> p c m", p=128)
                if jt == 0:
                    # K/V first (need only wk/wv/xt); wq streams meanwhile
                    nc.sync.dma_start(out=wq_lo, in_=wq_r[:, 0:KD // 2, :])
                    nc.sync.dma_start(out=wq_hi, in_=wq_r[:, KD // 2:KD, :])
                    nc.sync.dma_start(out=msk_sb, in_=cmask[:, :, :])
                    for h in range(NHL, NHL + NKVL):
                        emit_p1_head(jt, h, QT)
                    for s in range(4):
                        emit_p1_v(jt, s)
                    for h in range(NHL):
                        emit_p1_head(jt, h, QT)
                    # wo only needed from o_proj(0), deferred to next block
                    nc.sync.dma_start(
                        out=wo_sb,
                        in_=wo[:, :].rearrange("(c p) n -> p c n", p=128))
                else:
                    for h in range(NHL + NKVL):
                        emit_p1_head(jt, h, QT)
                    for s in range(4):
                        emit_p1_v(jt, s)

                # deferred stage-B tails (heads 6,7) + o_proj of the
                # PREVIOUS block: their ACT deps completed during P1 above
                if pend is not None:
                    b6, b7, pot, pjq = pend
                    emit_tail_b(*b6)
                    emit_tail_b(*b7)
                    emit_oproj(pjq, pot)
                    pend = None

                # ---------------- Phase 2: attention for jq = jt -----
                jq = jt
                nchunks = 4 * jq + 4
                OT = otpool.tile([128, NHL, TQ], bf16, tag="OT")
                penda = None  # head awaiting stage A (one-head deferral)
                pendb = None  # head awaiting stage B (two-head deferral)
                for h in range(NHL):
                    g = h // 4
                    o_ps = opool.tile([128, TQ], f32, tag="o")
                    den_ps = nrm.tile([1, TQ], f32, tag="nrm")
                    for c in range(nchunks):
                        r = c - 4 * jq
                        # diagonal-crossing chunk r: columns q < 128r are
                        # fully masked — skip them in S/exp/O/den entirely
                        q0 = 128 * r if r > 0 else 0
                        s_ps = spool.tile([128, TQ], f32, tag="s")
                        nc.tensor.matmul(s_ps[:, q0:TQ],
                                         lhsT=KT_sb[:, g, c * 128:(c + 1) * 128],
                                         rhs=QT[:, h, q0:TQ],
                                         start=True, stop=True)
                        p = ppool.tile([128, TQ], bf16, tag="p")
                        nc.scalar.activation(p[:, q0:TQ], s_ps[:, q0:TQ],
                                             EXP, scale=SCALE)
                        if r >= 0:  # mask the 128-wide triangle block
                            nc.vector.tensor_mul(p[:, q0:q0 + 128],
                                                 p[:, q0:q0 + 128],
                                                 msk_sb[:, r, q0:q0 + 128])
                        nc.tensor.matmul(o_ps[:, q0:TQ],
                                         lhsT=V_sb[:, c, g * 128:(g + 1) * 128],
                                         rhs=p[:, q0:TQ],
                                         start=(c == 0), stop=(c == nchunks - 1))
                        nc.tensor.matmul(den_ps[:, q0:TQ], lhsT=ones_bf,
                                         rhs=p[:, q0:TQ],
                                         start=(c == 0), stop=(c == nchunks - 1))
                    if pendb is not None:
                        emit_tail_b(*pendb)
                        pendb = None
                    if penda is not None:
                        pendb = emit_tail_a(*penda)
                    penda = (o_ps, den_ps, OT[:, h, :])
                # flush: A for head 7 now; B for heads 6,7 + o_proj are
                # deferred past the next block's P1
                b7 = emit_tail_a(*penda)
                pend = (pendb, b7, OT, jq)

            # last block's deferred stage-B tails + o_proj
            b6, b7, pot, pjq = pend
            emit_tail_b(*b6)
            emit_tail_b(*b7)
            emit_oproj(pjq, pot)
    return nc


def rope_tables(T=2048):
    inv = 1.0 / (THETA ** (np.arange(0, HD, 2, dtype=np.float32) / HD))
    t = np.arange(T, dtype=np.float32)
    freqs = np.outer(t, inv)
    emb = np.concatenate([freqs, freqs], -1)      # [T, 128]
    bf = ml_dtypes.bfloat16
    cos = np.ascontiguousarray(np.cos(emb).T.astype(bf))
    sin = np.sin(emb).T.astype(np.float32)
    sin_signed = sin.copy()
    sin_signed[:64] *= -1.0                        # rotate_half sign fold
    return cos, np.ascontiguousarray(sin_signed.astype(bf))


def causal_block_masks():
    k = np.arange(128)[:, None]
    q = np.arange(TQ)[None, :]
    cm = np.stack([(k + 128 * r) <= q for r in range(4)], axis=1)
    return np.ascontiguousarray(cm.astype(ml_dtypes.bfloat16))  # [128, 4, TQ]


def build_in_maps(x, wq, wk, wv, wo, T=2048):
    bf = ml_dtypes.bfloat16
    cos, sin_s = rope_tables(T)
    cm = causal_block_masks()
    wq16 = np.asarray(wq).astype(bf)
    wk16 = np.asarray(wk).astype(bf)
    wv16 = np.asarray(wv).astype(bf)
    wo16 = np.asarray(wo).astype(bf)
    in_maps = []
    for core in range(NCORES):
        b, hg = core // 2, core % 2
        in_maps.append({
            "xT": np.ascontiguousarray(np.asarray(x)[b].T).astype(bf),
            "wq": np.ascontiguousarray(wq16[:, hg * DQ:(hg + 1) * DQ]),
            "wk": np.ascontiguousarray(wk16[:, hg * DKV:(hg + 1) * DKV]),
            "wv": np.ascontiguousarray(wv16[:, hg * DKV:(hg + 1) * DKV]),
            "wo": np.ascontiguousarray(wo16[hg * DQ:(hg + 1) * DQ, :]),
            "cosT": cos, "sinT": sin_s, "cmask": cm,
        })
    return in_maps


_NC_CACHE = {}


def get_nc(T=2048):
    if T not in _NC_CACHE:
        _NC_CACHE[T] = build_nc(T)
    return _NC_CACHE[T]


def run(inputs, trace=False, **kw):
    """Returns (full_output [B,T,D] f32, BassKernelResults)."""
    from concourse import bass_utils
    x = np.asarray(inputs["x"], dtype=np.float32)
    T = x.shape[1]
    nc = get_nc(T)
    in_maps = build_in_maps(x, inputs["wq"], inputs["wk"], inputs["wv"],
                            inputs["wo"], T)
    res = bass_utils.run_bass_kernel_spmd(nc, in_maps,
                                          core_ids=list(range(NCORES)),
                                          trace=trace, **kw)
    outs = [np.asarray(r["out"]) for r in res.results]
    full = np.empty((B, T, D), dtype=np.float32)
    for b in range(B):
        full[b] = outs[2 * b].astype(np.float32) + outs[2 * b + 1].astype(np.float32)
    return full, res


def kernel(x, mask, wq, wk, wv, wo):
    full, _ = run({"x": x, "mask": mask, "wq": wq, "wk": wk, "wv": wv, "wo": wo})
    return full


# revision 45
# speedup vs baseline: 1.2471x; 1.0351x over previous
"""GQA attention kernel for Trainium2, 8 NeuronCores.

Sharding: data-parallel over batch (4) x tensor-parallel over head groups (2).
Each core handles one (batch, head-group): 8 query heads / 2 kv heads.
o_proj is row-parallel -> host sums the 2 partial outputs per batch.

v7 layout strategy (per core):
  - Inputs host-prepped: xT = x[b].T (bf16), weight shards (bf16),
    RoPE tables cosT/sinT [128, T] (bf16, sin sign-folded), causal masks.
  - Program order interleaves phases per 512-wide tile jt; the in-order
    PE stream never waits on ACT/DVE chains: each head's normalize tail
    is split in two stages pipelined across heads, and the last heads'
    tails + o_proj are deferred past the next block's P1.
  - P1: QT/KT via lhsT=w chunks (RoPE via SBUF shift-DMA + DVE, bf16);
    V in natural [T, dh] layout.
  - P2 per head, per 128-k chunk: S^T = K Q^T (PE); exp on ScalarE
    (PSUM->SBUF bf16, scaled); 0/1 mask on diagonal chunks (DVE);
    O^T += V^T P^T and denominator += ones^T P^T (PE, 3 PSUM banks).
    Tail stage A (one head later): unnormalized O^T to SBUF (DVE),
    1/den = exp(-ln(den)) on ScalarE (same table set as exp).
    Tail stage B (two heads later): broadcast 1/den via bf16 outer
    product (PE), scale O^T (DVE).
  - o_proj: lhsT=O^T slices, bf16 output; host sums partials in f32.
"""

import json as _json

import numpy as np
import ml_dtypes

import concourse.bass as bass
import concourse.mybir as mybir
import concourse.tile as tile

# --- walrus sync-wait legalizer -------------------------------------------
# The walrus build in this container encodes at most ONE sync-wait command
# per instruction ("Too many sync wait commands" in setupSyncWait<> for any
# instruction with 2+ waits, including Tile's own tail Drain). Legalize by
# splitting extra waits into standalone single-wait EventSemaphore
# instructions on the same engine, immediately before the instruction —
# identical semantics (the engine stalls on each wait in turn).

_MAX_WAITS = 1
_orig_to_json_bytes = bass.Bass.to_json_bytes


def _split_waits_json(raw: bytes) -> bytes:
    m = _json.loads(raw)
    changed = False
    for fn in m.get("functions", []):
        for bb in fn.get("blocks", []):
            out = []
            for inst in bb.get("instructions", []):
                si = inst.get("sync_info")
                waits = (si or {}).get("on_wait") or []
                if len(waits) > _MAX_WAITS:
                    changed = True
                    for k, w in enumerate(waits[:-_MAX_WAITS]):
                        out.append({
                            "debug": inst.get("debug", 0),
                            "engine": inst["engine"],
                            "ins": [], "outs": [],
                            "name": f"{inst['name']}-sw{k}",
                            "opcode": "EventSemaphore",
                            "sync_info": {"on_update": [], "on_wait": [w]},
                        })
                    si["on_wait"] = waits[-_MAX_WAITS:]
                out.append(inst)
            bb["instructions"] = out
    if not changed:
        return raw
    return _json.dumps(m).encode()


def _patched_to_json_bytes(self):
    return _split_waits_json(_orig_to_json_bytes(self))


bass.Bass.to_json_bytes = _patched_to_json_bytes
# --------------------------------------------------------------------------

B, D = 4, 2048
NH, NKV, HD = 16, 4, 128
NHL, NKVL = 8, 2          # per-core q heads / kv heads
DQ = NHL * HD             # 1024
DKV = NKVL * HD           # 256
KD = D // 128             # 16 contraction chunks
TQ = 512                  # query tile width (matmul free dim)
THETA = 10000.0
SCALE = HD ** -0.5
NCORES = 8

bf16 = mybir.dt.bfloat16
f32 = mybir.dt.float32
EXP = mybir.ActivationFunctionType.Exp
LN = mybir.ActivationFunctionType.Ln


def build_nc(T=2048):
    njq = T // TQ
    nck = T // 128
    ts = bass.ts

    nc = bass.Bass()
    xT = nc.dram_tensor("xT", [D, T], bf16, kind="ExternalInput")
    wq = nc.dram_tensor("wq", [D, DQ], bf16, kind="ExternalInput")
    wk = nc.dram_tensor("wk", [D, DKV], bf16, kind="ExternalInput")
    wv = nc.dram_tensor("wv", [D, DKV], bf16, kind="ExternalInput")
    wo = nc.dram_tensor("wo", [DQ, D], bf16, kind="ExternalInput")
    cosT = nc.dram_tensor("cosT", [HD, T], bf16, kind="ExternalInput")
    sinT = nc.dram_tensor("sinT", [HD, T], bf16, kind="ExternalInput")
    cmask = nc.dram_tensor("cmask", [128, 4, TQ], bf16, kind="ExternalInput")
    out = nc.dram_tensor("out", [T, D], bf16, kind="ExternalOutput")

    with tile.TileContext(nc) as tc:
        with tc.tile_pool(name="res", bufs=1) as res, \
             tc.tile_pool(name="wts", bufs=1) as wts, \
             tc.tile_pool(name="qt", bufs=2) as qtpool, \
             tc.tile_pool(name="xp", bufs=1) as xpool, \
             tc.tile_pool(name="tp", bufs=2) as tpool, \
             tc.tile_pool(name="gp", bufs=2, space="PSUM") as gp, \
             tc.tile_pool(name="sp", bufs=2, space="PSUM") as spool, \
             tc.tile_pool(name="op", bufs=2, space="PSUM") as opool, \
             tc.tile_pool(name="nrm", bufs=2, space="PSUM") as nrm, \
             tc.tile_pool(name="pp", bufs=6) as ppool, \
             tc.tile_pool(name="dg", bufs=6) as dgpool, \
             tc.tile_pool(name="sm", bufs=2) as smpool, \
             tc.tile_pool(name="oc", bufs=3) as ocpool, \
             tc.tile_pool(name="bc", bufs=2) as bcpool, \
             tc.tile_pool(name="ot", bufs=2) as otpool, \
             tc.tile_pool(name="ou", bufs=2) as outpool:

            KT_sb = res.tile([128, NKVL, T], bf16)
            V_sb = res.tile([128, nck, DKV], bf16)
            msk_sb = res.tile([128, 4, TQ], bf16)
            ones_bf = res.tile([128, 1], bf16)
            onesr_bf = res.tile([1, 128], bf16)
            dum_i = res.tile([1, 16], f32)
            dum_o = res.tile([1, 16], bf16)

            wq_lo = wts.tile([128, KD // 2, DQ], bf16)
            wq_hi = wts.tile([128, KD // 2, DQ], bf16)
            wk_sb = wts.tile([128, KD, DKV], bf16)
            wv_sb = wts.tile([128, KD, DKV], bf16)
            wo_sb = wts.tile([128, NHL, D], bf16)
            cos_sb = wts.tile([128, T], bf16)
            sin_sb = wts.tile([128, T], bf16)

            # preload the ln/exp table set right away (ACT is idle anyway)
            nc.vector.memset(dum_i, 1.0)
            nc.scalar.activation(dum_o, dum_i, EXP)
            # first-wave DMAs: what P1(jt=0)'s K/V heads need comes first
            nc.sync.dma_start(out=wk_sb, in_=wk[:, :].rearrange("(c p) m -> p c m", p=128))
            nc.sync.dma_start(out=wv_sb, in_=wv[:, :].rearrange("(c p) m -> p c m", p=128))
            nc.sync.dma_start(out=cos_sb, in_=cosT[:, :])
            nc.sync.dma_start(out=sin_sb, in_=sinT[:, :])
            nc.vector.memset(ones_bf, 1.0)
            nc.vector.memset(onesr_bf, 1.0)

            # Two-stage normalize tail, pipelined across heads so the PE
            # matmuls in it never wait on the ACT ln/exp chain.
            def emit_tail_a(o_ps, den_ps, ot_dst):
                """Free o_ps into SBUF; 1/den via exp(-ln) on ACT."""
                ocp = ocpool.tile([128, TQ], bf16, tag="ocp")
                nc.vector.tensor_copy(ocp, o_ps)
                lden = smpool.tile([1, TQ], f32, tag="lden")
                nc.scalar.activation(lden, den_ps, LN)
                rden = smpool.tile([1, TQ], bf16, tag="rden")
                nc.scalar.activation(rden, lden, EXP, scale=-1.0)
                return (ocp, rden, ot_dst)

            def emit_tail_b(ocp, rden, ot_dst):
                """Broadcast 1/den across partitions (outer product), scale."""
                bc_ps = gp.tile([128, TQ], f32, tag="gp")
                nc.tensor.matmul(bc_ps, lhsT=onesr_bf, rhs=rden,
                                 start=True, stop=True)
                bc_sb = bcpool.tile([128, TQ], bf16, tag="bc")
                nc.vector.tensor_copy(bc_sb, bc_ps)
                nc.vector.tensor_mul(ot_dst, ocp, bc_sb)

            def emit_oproj(jq, OT):
                for s in range(4):
                    row = jq * TQ + s * 128
                    for half in range(2):
                        osb = outpool.tile([128, D // 2], bf16, tag="osb")
                        for nt in range(2):
                            ntg = half * 2 + nt
                            op_ps = gp.tile([128, TQ], f32, tag="gp")
                            for hc in range(NHL):
                                nc.tensor.matmul(
                                    op_ps,
                                    lhsT=OT[:, hc, s * 128:(s + 1) * 128],
                                    rhs=wo_sb[:, hc, ts(ntg, TQ)],
                                    start=(hc == 0), stop=(hc == NHL - 1))
                            nc.scalar.copy(osb[:, ts(nt, TQ)], op_ps)
                        nc.sync.dma_start(
                            out=out[row:row + 128, half * (D // 2):(half + 1) * (D // 2)],
                            in_=osb)

            def emit_p1_head(jt, h, QT):
                """Projection + RoPE for one head (h<NHL: Q, else K)."""
                if h < NHL:
                    col = h * 128
                    dst = QT[:, h, :]
                else:
                    g = h - NHL
                    col = g * 128
                    dst = KT_sb[:, g, ts(jt, TQ)]
                ps = gp.tile([128, TQ], f32, tag="gp")
                for c in range(KD):
                    if h < NHL:
                        w_sb = wq_lo if c < KD // 2 else wq_hi
                        ci = c % (KD // 2)
                    else:
                        w_sb, ci = wk_sb, c
                    nc.tensor.matmul(ps, lhsT=w_sb[:, ci, col:col + 128],
                                     rhs=xt_cur[c // 4][:, c % 4, :],
                                     start=(c == 0), stop=(c == KD - 1))
                qf = tpool.tile([128, TQ], bf16, tag="qf")
                nc.vector.tensor_copy(qf, ps)
                qs = tpool.tile([128, TQ], bf16, tag="qs")
                nc.sync.dma_start(out=qs[0:64, :], in_=qf[64:128, :])
                nc.sync.dma_start(out=qs[64:128, :], in_=qf[0:64, :])
                t1 = tpool.tile([128, TQ], bf16, tag="t1")
                nc.vector.tensor_mul(t1, qf, cos_sb[:, ts(jt, TQ)])
                nc.vector.tensor_mul(qs, qs, sin_sb[:, ts(jt, TQ)])
                nc.vector.tensor_add(dst, t1, qs)

            def emit_p1_v(jt, s):
                pv = gp.tile([128, TQ], f32, tag="gp")
                for c in range(KD):
                    nc.tensor.matmul(pv[:, 0:DKV],
                                     lhsT=xt_cur[c // 4][:, c % 4, s * 128:(s + 1) * 128],
                                     rhs=wv_sb[:, c, :],
                                     start=(c == 0), stop=(c == KD - 1))
                nc.vector.tensor_copy(V_sb[:, 4 * jt + s, :], pv[:, 0:DKV])

            xT_r = xT[:, :].rearrange("(c p) t -> p c t", p=128)
            xt_cur = [None] * 4
            pend = None  # deferred stage-B tails + o_proj carry
            for jt in range(njq):
                # ---------------- Phase 1 for tile jt ----------------
                # xt in 4 sub-tiles: the first matmul only waits on 512 KB
                for q4 in range(4):
                    xq = xpool.tile([128, 4, TQ], bf16, tag=f"xt{q4}")
                    xt_cur[q4] = xq
                    nc.sync.dma_start(
                        out=xq, in_=xT_r[:, 4 * q4:4 * q4 + 4, ts(jt, TQ)])
                QT = qtpool.tile([128, NHL, TQ], bf16, tag="QT")
                wq_r = wq[:, :].rearrange("(c p) m -> p c m", p=128)
                if jt == 0:
                    # K/V first (need only wk/wv/xt); wq streams meanwhile
                    nc.sync.dma_start(out=wq_lo, in_=wq_r[:, 0:KD // 2, :])
                    nc.sync.dma_start(out=wq_hi, in_=wq_r[:, KD // 2:KD, :])
                    nc.sync.dma_start(out=msk_sb, in_=cmask[:, :, :])
                    for h in range(NHL, NHL + NKVL):
                        emit_p1_head(jt, h, QT)
                    for s in range(4):
                        emit_p1_v(jt, s)
                    for h in range(NHL):
                        emit_p1_head(jt, h, QT)
                    # wo only needed from o_proj(0), deferred to next block
                    nc.sync.dma_start(
                        out=wo_sb,
                        in_=wo[:, :].rearrange("(c p) n -> p c n", p=128))
                else:
                    for h in range(NHL + NKVL):
                        emit_p1_head(jt, h, QT)
                    for s in range(4):
                        emit_p1_v(jt, s)

                # deferred stage-B tails (heads 6,7) + o_proj of the
                # PREVIOUS block: their ACT deps completed during P1 above
                if pend is not None:
                    b6, b7, pot, pjq = pend
                    emit_tail_b(*b6)
                    emit_tail_b(*b7)
                    emit_oproj(pjq, pot)
                    pend = None

                # ---------------- Phase 2: attention for jq = jt -----
                jq = jt
                nchunks = 4 * jq + 4
                OT = otpool.tile([128, NHL, TQ], bf16, tag="OT")
                penda = None  # head awaiting stage A (one-head deferral)
                pendb = None  # head awaiting stage B (two-head deferral)
                for h in range(NHL):
                    g = h // 4
                    o_ps = opool.tile([128, TQ], f32, tag="o")
                    plist = []   # p tiles of the 4 diagonal chunks
                    dgts = []    # bf16 pair-sums of non-diagonal chunk pairs
                    prev_p = None
                    for c in range(nchunks):
                        r = c - 4 * jq
                        # diagonal-crossing chunk r: columns q < 128r are
                        # fully masked — skip them in S/exp/O/den entirely
                        q0 = 128 * r if r > 0 else 0
                        s_ps = spool.tile([128, TQ], f32, tag="s")
                        nc.tensor.matmul(s_ps[:, q0:TQ],
                                         lhsT=KT_sb[:, g, c * 128:(c + 1) * 128],
                                         rhs=QT[:, h, q0:TQ],
                                         start=True, stop=True)
                        p = ppool.tile([128, TQ], bf16, tag="p")
                        nc.scalar.activation(p[:, q0:TQ], s_ps[:, q0:TQ],
                                             EXP, scale=SCALE)
                        if r >= 0:  # mask the 128-wide triangle block
                            nc.vector.tensor_mul(p[:, q0:q0 + 128],
                                                 p[:, q0:q0 + 128],
                                                 msk_sb[:, r, q0:q0 + 128])
                            plist.append((p, q0))
                        elif c % 2 == 0:
                            prev_p = p
                        else:
                            dgt = dgpool.tile([128, TQ], bf16, tag="dg")
                            nc.vector.tensor_add(dgt, prev_p, p)
                            dgts.append(dgt)
                        nc.tensor.matmul(o_ps[:, q0:TQ],
                                         lhsT=V_sb[:, c, g * 128:(g + 1) * 128],
                                         rhs=p[:, q0:TQ],
                                         start=(c == 0), stop=(c == nchunks - 1))
                    # denominator matmuls, emitted densely at head end
                    # (pair-sums and diagonal p tiles are all ready by now)
                    den_ps = nrm.tile([1, TQ], f32, tag="nrm")
                    nd = len(dgts) + len(plist)
                    di = 0
                    for dgt in dgts:
                        nc.tensor.matmul(den_ps, lhsT=ones_bf, rhs=dgt,
                                         start=(di == 0), stop=(di == nd - 1))
                        di += 1
                    for p, q0 in plist:
                        nc.tensor.matmul(den_ps[:, q0:TQ], lhsT=ones_bf,
                                         rhs=p[:, q0:TQ],
                                         start=(di == 0), stop=(di == nd - 1))
                        di += 1
                    if pendb is not None:
                        emit_tail_b(*pendb)
                        pendb = None
                    if penda is not None:
                        pendb = emit_tail_a(*penda)
                    penda = (o_ps, den_ps, OT[:, h, :])
                # flush: A for head 7 now; B for heads 6,7 + o_proj are
                # deferred past the next block's P1
                b7 = emit_tail_a(*penda)
                pend = (pendb, b7, OT, jq)

            # last block's deferred stage-B tails + o_proj
            b6, b7, pot, pjq = pend
            emit_tail_b(*b6)
            emit_tail_b(*b7)
            emit_oproj(pjq, pot)
    return nc


def rope_tables(T=2048):
    inv = 1.0 / (THETA ** (np.arange(0, HD, 2, dtype=np.float32) / HD))
    t = np.arange(T, dtype=np.float32)
    freqs = np.outer(t, inv)
    emb = np.concatenate([freqs, freqs], -1)      # [T, 128]
    bf = ml_dtypes.bfloat16
    cos = np.ascontiguousarray(np.cos(emb).T.astype(bf))
    sin = np.sin(emb).T.astype(np.float32)
    sin_signed = sin.copy()
    sin_signed[:64] *= -1.0                        # rotate_half sign fold
    return cos, np.ascontiguousarray(sin_signed.astype(bf))


def causal_block_masks():
    k = np.arange(128)[:, None]
    q = np.arange(TQ)[None, :]
    cm = np.stack([(k + 128 * r) <= q for r in range(4)], axis=1)
    return np.ascontiguousarray(cm.astype(ml_dtypes.bfloat16))  # [128, 4, TQ]


def build_in_maps(x, wq, wk, wv, wo, T=2048):
    bf = ml_dtypes.bfloat16
    cos, sin_s = rope_tables(T)
    cm = causal_block_masks()
    wq16 = np.asarray(wq).astype(bf)
    wk16 = np.asarray(wk).astype(bf)
    wv16 = np.asarray(wv).astype(bf)
    wo16 = np.asarray(wo).astype(bf)
    in_maps = []
    for core in range(NCORES):
        b, hg = core // 2, core % 2
        in_maps.append({
            "xT": np.ascontiguousarray(np.asarray(x)[b].T).astype(bf),
            "wq": np.ascontiguousarray(wq16[:, hg * DQ:(hg + 1) * DQ]),
            "wk": np.ascontiguousarray(wk16[:, hg * DKV:(hg + 1) * DKV]),
            "wv": np.ascontiguousarray(wv16[:, hg * DKV:(hg + 1) * DKV]),
            "wo": np.ascontiguousarray(wo16[hg * DQ:(hg + 1) * DQ, :]),
            "cosT": cos, "sinT": sin_s, "cmask": cm,
        })
    return in_maps


_NC_CACHE = {}


def get_nc(T=2048):
    if T not in _NC_CACHE:
        _NC_CACHE[T] = build_nc(T)
    return _NC_CACHE[T]


def run(inputs, trace=False, **kw):
    """Returns (full_output [B,T,D] f32, BassKernelResults)."""
    from concourse import bass_utils
    x = np.asarray(inputs["x"], dtype=np.float32)
    T = x.shape[1]
    nc = get_nc(T)
    in_maps = build_in_maps(x, inputs["wq"], inputs["wk"], inputs["wv"],
                            inputs["wo"], T)
    res = bass_utils.run_bass_kernel_spmd(nc, in_maps,
                                          core_ids=list(range(NCORES)),
                                          trace=trace, **kw)
    outs = [np.asarray(r["out"]) for r in res.results]
    full = np.empty((B, T, D), dtype=np.float32)
    for b in range(B):
        full[b] = outs[2 * b].astype(np.float32) + outs[2 * b + 1].astype(np.float32)
    return full, res


def kernel(x, mask, wq, wk, wv, wo):
    full, _ = run({"x": x, "mask": mask, "wq": wq, "wk": wk, "wv": wv, "wo": wo})
    return full
